# revision 47
# baseline (speedup 1.0000x reference)
"""Trainium2 Bass kernel: sparse AE encoder (L1 fan-in-1 -> relu/BN -> L2 block-diag
4x4 -> relu/BN -> L3 sparse 256-nnz/TF -> BN), SPMD over 8 NeuronCores.

Sharding: hidden axis across cores (BN1/BN2 local; one z-reduce at the end).
Host packs w1*x+b1 into the hidden-row x layout so L1 is a bare batched relu.
L2 runs on the TensorEngine with block-diagonal [128,128] bf16 stationaries,
L3 as dense matmul against the host-densified W3 shard (batch-half stationary,
w3 streaming, N=512 PSUM accumulators over all 32 tiles).

Schedule notes (v14):
- PE warmup burst (identity matmuls) during the input-DMA phase nudges the
  HAM clock gate toward 2.4 GHz before real matmuls start.
- norm_params does the bn_stats half-combine on [128,8] groups (GB=8), with
  1/sqrt as Scalar-Sqrt + Vector-reciprocal (2 ops instead of a 6-op Newton
  chain).  h1n normalize on GpSimd, h2n split Scalar/Vector using
  relu(istd*h) == istd*h (h = relu output >= 0, istd > 0) so Scalar's Relu
  with a scale AP does the scale-only normalize.  NEVER split one pass
  across Vector AND GpSimd: DVE 2-port mode locks GpSimd out of their shared
  SBUF port (measured 3.9us per op vs 560ns).
- ONE AllToAll for the z partials (batch-major layout, 2 PE transposes after
  the local tree-reduce instead of 16 before).  The collective firmware only
  becomes responsive ~60-67us after kernel start and serializes collectives,
  so chunked/pipelined exchanges buy nothing; a tiny sacrificial AllToAll
  (with its input actually written - an unwritten input makes the Tile
  scheduler classify the collective as never-ready and emit it LAST) absorbs
  the ncfw wake + cold-mesh pass while compute runs.
"""

import numpy as np
import ml_dtypes

import concourse.bacc as bacc
import concourse.bass as bass
import concourse.tile as tile
import concourse.mybir as mybir
from concourse import bass_utils
from concourse.masks import make_identity

N_GENES = 8192
WM = 4
HID = N_GENES * WM          # 32768
N_TF = 1024
B = 256
EPS = 1e-5

NCORES = 8
HSH = HID // NCORES         # 4096 hidden rows / core
P = 128
NT = HSH // P               # 32 hidden tiles / core
GB = 8                      # group size (tiles)
NG = NT // GB               # 4 groups
HT = NT // 2                # 16 tiles per z-chunk

BF16 = ml_dtypes.bfloat16
F32 = mybir.dt.float32
F16 = mybir.dt.float16
BF = mybir.dt.bfloat16
I32 = mybir.dt.int32
AF = mybir.ActivationFunctionType
OP = mybir.AluOpType

TRACE = False
LAST_RESULT = None

_cache = {}


def _build_graph():
    nc = bacc.Bacc("TRN2", target_bir_lowering=False, debug=False, num_devices=NCORES)

    xrd = nc.dram_tensor("xrd", [P, NT * B], BF, kind="ExternalInput").ap()
    w2d = nc.dram_tensor("w2d", [P, NT * P], BF, kind="ExternalInput").ap()
    w3d = nc.dram_tensor("w3d", [P, NT * N_TF], BF, kind="ExternalInput").ap()
    b2d = nc.dram_tensor("b2d", [P, NT], F32, kind="ExternalInput").ap()
    outT = nc.dram_tensor("outT", [P, B], F32, kind="ExternalOutput").ap()

    from contextlib import ExitStack
    with tile.TileContext(nc) as tc, ExitStack() as ctx:
        cpool = ctx.enter_context(tc.tile_pool(name="const", bufs=1))
        wpool = ctx.enter_context(tc.tile_pool(name="wts", bufs=1))
        apool = ctx.enter_context(tc.tile_pool(name="acts", bufs=1))
        spool = ctx.enter_context(tc.tile_pool(name="stats", bufs=4))
        zcpool = ctx.enter_context(tc.tile_pool(name="zchunk", bufs=1))
        psZp = ctx.enter_context(tc.tile_pool(name="psZ", bufs=1, space="PSUM"))
        psLp = ctx.enter_context(tc.tile_pool(name="psL", bufs=2, space="PSUM"))
        psWp = ctx.enter_context(tc.tile_pool(name="psW", bufs=1, space="PSUM"))
        psTp = ctx.enter_context(tc.tile_pool(name="psT", bufs=1, space="PSUM"))
        dpool = ctx.enter_context(tc.tile_pool(name="dram", bufs=1, space="DRAM"))

        # ---- warmup collective (slot #2 after the bass prelude barrier):
        # absorbs the ncfw wakeup + cold-mesh pass while compute runs, so the
        # real AllToAll runs warm.  Its input must actually be written or the
        # Tile scheduler emits the collective last.
        dum_in = dpool.tile([NCORES, NCORES], F16, name="dum_in")
        dum_out = dpool.tile([NCORES, NCORES], F16, name="dum_out")
        dum_sb = cpool.tile([NCORES, NCORES], F16, name="dum_sb")
        nc.gpsimd.memset(dum_sb[:], 0)
        nc.sync.dma_start(dum_in[:], dum_sb[:])
        nc.gpsimd.collective_compute(
            "AllToAll", OP.bypass, replica_groups=[list(range(NCORES))],
            ins=[dum_in.opt()], outs=[dum_out.opt()])

        # ---- static loads, issued in compute-consumption order -----------
        b2t = cpool.tile([P, NT], F32, name="b2t")
        nc.sync.dma_start(b2t[:], b2d[:])

        # Force the sqrt table set to load up front: Relu/Copy are filler
        # functions present in every set, so after this single load no other
        # ACT_TABLE_LOAD is needed.
        sqwt = cpool.tile([P, 1], F32, name="sqwt")
        nc.gpsimd.memset(sqwt[:], 1.0)
        nc.scalar.activation(sqwt[:], sqwt[:], AF.Sqrt)
        epst = cpool.tile([P, 1], F32, name="epst")
        nc.gpsimd.memset(epst[:], EPS)

        xrep = wpool.tile([P, NT * B], BF, name="xrep")
        w2s = wpool.tile([P, NT * P], BF, name="w2s")
        w3s = wpool.tile([P, NT * N_TF], BF, name="w3s")

        def load_x(c):     # 4 chunks of 8 tiles each (512KB)
            xcw = GB * B
            nc.sync.dma_start(xrep[:, c * xcw:(c + 1) * xcw],
                              xrd[:, c * xcw:(c + 1) * xcw])

        def load_w2(c):    # 2 chunks of 16 tiles each (512KB)
            cw = 16 * P
            nc.sync.dma_start(w2s[:, c * cw:(c + 1) * cw],
                              w2d[:, c * cw:(c + 1) * cw])

        def load_w3(c):    # 8 chunks of 4 tiles each (1MB)
            cw = 4 * N_TF
            nc.sync.dma_start(w3s[:, c * cw:(c + 1) * cw],
                              w3d[:, c * cw:(c + 1) * cw])

        load_x(0); load_w2(0); load_x(1)
        load_w3(0); load_w3(1)
        load_x(2); load_w2(1); load_w3(2); load_w3(3)
        load_x(3); load_w3(4); load_w3(5)
        load_w3(6); load_w3(7)

        idt = cpool.tile([P, P], F16, name="idt")
        make_identity(nc, idt[:])

        # ---- PE warmup burst: hold the HAM clock gate at 8/8 -------------
        # The PE is idle during the input-DMA phase; a stream of junk
        # identity matmuls warms the clock gate (~3.4us busy window) so the
        # real L2/L3 matmuls run at 2.4 GHz instead of 1.2.
        NWARM = 56
        psW = psWp.tile([P, P], F32, name="psW", tag="psW")
        for i in range(NWARM):
            nc.tensor.matmul(psW[:], lhsT=idt[:], rhs=idt[:],
                             start=True, stop=True, skip_group_check=True)

        def rsqrt_sv(istd, var, w=GB):
            """istd = 1/sqrt(var): Sqrt on Scalar (spline) + reciprocal on
            Vector (full-precision HW divide) — 2 ops instead of a 6-op
            Newton chain on Vector."""
            sq = spool.tile([P, GB], F32, name="sq", tag="sq")
            nc.scalar.activation(sq[:, :w], var, AF.Sqrt)
            nc.vector.reciprocal(istd, sq[:, :w])

        hrA = apool.tile([P, NT * B], BF, name="hrA")
        h1n = apool.tile([P, NT * B], BF, name="h1n")
        hrB = apool.tile([P, NT * B], BF, name="hrB")
        h2n = apool.tile([P, NT * B], BF, name="h2n")
        st1 = apool.tile([P, NT * 6], F32, name="st1")
        st2 = apool.tile([P, NT * 6], F32, name="st2")

        # layer-3 psums, accumulated across HT tiles per chunk
        psZ = [[psZp.tile([P, 512], F32, name=f"psZ{bh}{th}", tag=f"psZ{bh}{th}")
                for th in range(2)] for bh in range(2)]

        def norm_params(st, g0, istd, nm):
            """bn_stats 6-tuples (even/odd column halves) -> istd [, nm]."""
            sv = st[:, g0 * 6:(g0 + GB) * 6].rearrange("p (t s) -> p t s", s=6)
            me, mo = sv[:, :, 1], sv[:, :, 4]
            M2e, M2o = sv[:, :, 2], sv[:, :, 5]
            dm = spool.tile([P, GB], F32, name="dm", tag="dm")
            nc.vector.tensor_tensor(dm[:], me, mo, op=OP.subtract)
            vq = spool.tile([P, GB], F32, name="vq", tag="vq")
            nc.vector.scalar_tensor_tensor(vq[:], in0=dm[:], scalar=0.25,
                                           in1=dm[:], op0=OP.mult, op1=OP.mult)
            var = spool.tile([P, GB], F32, name="var", tag="var")
            nc.vector.tensor_tensor(var[:], M2e, M2o, op=OP.add)
            nc.vector.tensor_scalar(out=var[:], in0=var[:], scalar1=1.0 / B,
                                    scalar2=EPS, op0=OP.mult, op1=OP.add)
            nc.vector.tensor_tensor(var[:], var[:], vq[:], op=OP.add)
            rsqrt_sv(istd[:], var[:])
            if nm is not None:
                ms = spool.tile([P, GB], F32, name="ms", tag="ms")
                nc.vector.tensor_tensor(ms[:], me, mo, op=OP.add)
                nc.vector.scalar_tensor_tensor(nm[:], in0=ms[:], scalar=-0.5,
                                               in1=istd[:], op0=OP.mult,
                                               op1=OP.mult)

        def emitA(g):
            """L1 relu (Scalar, batched, pair-interleaved out) + BN1 stats
            (Vector, one per pair) + normalize (GpSimd/Vector split)."""
            t0 = g * GB
            for h in range(2):  # two [128, 4*B] relus per group
                lo = (t0 + 4 * h) * B
                nc.scalar.activation(hrA[:, lo:lo + 4 * B],
                                     xrep[:, lo:lo + 4 * B], AF.Relu)
            for t in range(t0, t0 + GB):
                nc.vector.bn_stats(st1[:, t * 6:(t + 1) * 6],
                                   hrA[:, t * B:(t + 1) * B])
            istd = spool.tile([P, GB], F32, name="istdA", tag="istdA")
            nm = spool.tile([P, GB], F32, name="nmA", tag="nmA")
            norm_params(st1, t0, istd, nm)
            for t in range(t0, t0 + GB):
                j = t - t0
                nc.gpsimd.tensor_scalar(out=h1n[:, t * B:(t + 1) * B],
                                        in0=hrA[:, t * B:(t + 1) * B],
                                        scalar1=istd[:, j:j + 1],
                                        scalar2=nm[:, j:j + 1],
                                        op0=OP.mult, op1=OP.add)

        def emitB_front(g):
            """L2 matmul (PE) + relu (Scalar, pair-interleaved out) + BN2
            stats (Vector, one per pair)."""
            t0 = g * GB
            for t in range(t0, t0 + GB):
                if t % 2 == 0:  # one PSUM bank holds two tiles' L2 outputs
                    ps2 = psLp.tile([P, 2 * B], F32, name="psL", tag="psL")
                ps = ps2[:, (t % 2) * B:(t % 2 + 1) * B]
                nc.tensor.matmul(ps, lhsT=w2s[:, t * P:(t + 1) * P],
                                 rhs=h1n[:, t * B:(t + 1) * B],
                                 start=True, stop=True, skip_group_check=True)
                nc.scalar.activation(hrB[:, t * B:(t + 1) * B], ps, AF.Relu,
                                     bias=b2t[:, t:t + 1])
                nc.vector.bn_stats(st2[:, t * 6:(t + 1) * 6],
                                   hrB[:, t * B:(t + 1) * B])

        def emitB_norm(g):
            """BN2 scale-only normalize (mean shift cancels in BN3)."""
            t0 = g * GB
            istd = spool.tile([P, GB], F32, name="istdB", tag="istdB")
            norm_params(st2, t0, istd, None)
            for t in range(t0, t0 + GB):
                j = t - t0
                if t % 2 == 0:
                    # relu(istd*h) == istd*relu(h) == istd*h for h = relu out,
                    # istd > 0 — lets Scalar do the scale-only normalize.
                    nc.scalar.activation(h2n[:, t * B:(t + 1) * B],
                                         hrB[:, t * B:(t + 1) * B], AF.Relu,
                                         scale=istd[:, j:j + 1])
                else:
                    nc.vector.tensor_scalar(out=h2n[:, t * B:(t + 1) * B],
                                            in0=hrB[:, t * B:(t + 1) * B],
                                            scalar1=istd[:, j:j + 1],
                                            scalar2=None, op0=OP.mult)

        # z accumulation chunk boundaries: first chunk is small (group 0
        # only) so its AllToAll triggers early and absorbs the cold-mesh cost
        # under compute; the later two run on a warm mesh.
        CH_START = {0}
        CH_STOP = {31}

        def emitL3(lo, hi):
            """z accumulation for tiles [lo, hi)."""
            for t in range(lo, hi):
                for bh in range(2):
                    for th in range(2):
                        nc.tensor.matmul(
                            psZ[bh][th][:],
                            lhsT=h2n[:, t * B + bh * P: t * B + (bh + 1) * P],
                            rhs=w3s[:, t * N_TF + th * 512: t * N_TF + (th + 1) * 512],
                            start=(t in CH_START), stop=(t in CH_STOP),
                            skip_group_check=True)

        # DRAM tensors for the two chunked AllToAlls.
        # zin layout: [j(rank) 8, bh 2, p 128, t 128] f16 -> rank j's slice is
        # rows [256j, 256j+256) = [bh*128+p, t].
        zin = [dpool.tile([NCORES * B, P], F16, name=f"zin{c}") for c in range(1)]
        za = [dpool.tile([NCORES * B, P], F16, name=f"za{c}") for c in range(1)]

        def drain_chunk(c):
            """psZ -> SBUF f16 -> DRAM zin[c] -> AllToAll trigger."""
            zc = zcpool.tile([P, 2 * N_TF], F16, name="zc", tag="zc")
            # zc columns: (bh, th*512+t~): Scalar does bh=0, Vector bh=1
            for th in range(2):
                nc.scalar.activation(zc[:, th * 512:(th + 1) * 512],
                                     psZ[0][th][:], AF.Copy)
                nc.vector.tensor_copy(zc[:, N_TF + th * 512: N_TF + (th + 1) * 512],
                                      psZ[1][th][:])
            zv = zin[c].rearrange("(j bh p) t -> p j bh t", j=NCORES, bh=2)
            for bh in range(2):
                nc.sync.dma_start(
                    zv[:, :, bh, :],
                    zc[:, bh * N_TF:(bh + 1) * N_TF].rearrange(
                        "p (j t) -> p j t", j=NCORES))
            nc.gpsimd.collective_compute(
                "AllToAll", OP.bypass, replica_groups=[list(range(NCORES))],
                ins=[zin[c].opt()], outs=[za[c].opt()])

        def reduce_chunk(c, acc_prev):
            """za[c] -> SBUF, reduce 16 received [128,128] blocks by summing
            column-halves: blocks are (rank, bh)-major, so 2048->1024->512->
            256 halvings reduce over ranks and land exactly in (bh, t)
            layout — 3 wide Vector ops instead of a 6-op tree."""
            assert acc_prev is None
            zs = zcpool.tile([P, NCORES * B], F16, name=f"zsum{c}", tag=f"zsum{c}")
            nc.scalar.dma_start(
                zs[:].rearrange("p (q t) -> p q t", q=2 * NCORES),
                za[c].rearrange("(q p) t -> p q t", p=P))
            t4 = zcpool.tile([P, 4 * B], F16, name=f"t4_{c}", tag="t4")
            nc.vector.tensor_tensor(t4[:], zs[:, 0:4 * B], zs[:, 4 * B:8 * B],
                                    op=OP.add)
            t2 = zcpool.tile([P, 2 * B], F16, name=f"t2_{c}", tag="t2")
            nc.vector.tensor_tensor(t2[:], t4[:, 0:2 * B], t4[:, 2 * B:4 * B],
                                    op=OP.add)
            red = zcpool.tile([P, B], F16, name=f"red{c}", tag=f"red{c}")
            nc.vector.tensor_tensor(red[:], t2[:, 0:B], t2[:, B:2 * B], op=OP.add)
            return red

        # ---- main pipeline ------------------------------------------------
        emitA(0)
        emitB_front(0); emitB_norm(0)
        emitA(1)
        emitB_front(1); emitB_norm(1)
        emitL3(0, 8)
        emitA(2)
        emitB_front(2); emitB_norm(2)
        emitL3(8, 16)
        emitA(3)
        emitB_front(3); emitB_norm(3)
        emitL3(16, 24)
        emitL3(24, 32)
        drain_chunk(0)
        redf = reduce_chunk(0, None)

        # ---- transpose to TF-major, BN3, output ---------------------------
        zfin = zcpool.tile([P, B], F16, name="zfin", tag="zfin")
        for bh in range(2):
            pst = psTp.tile([P, P], F16, name="pst", tag="pst")
            nc.tensor.transpose(pst[:], in_=redf[:, bh * P:(bh + 1) * P],
                                identity=idt[:])
            nc.scalar.activation(zfin[:, bh * P:(bh + 1) * P], pst[:], AF.Copy)

        st6 = spool.tile([P, 6], F32, name="st6", tag="st6")
        nc.vector.bn_stats(st6[:], zfin[:])
        mv3 = spool.tile([P, 2], F32, name="mv3", tag="mv3")
        nc.vector.bn_aggr(mv3[:], st6[:])
        # sqrt(var + EPS) with the +EPS folded into the activation bias
        sq3 = spool.tile([P, 1], F32, name="sq3", tag="sq3")
        nc.scalar.activation(sq3[:], mv3[:, 1:2], AF.Sqrt, bias=epst[:])
        istd3 = spool.tile([P, 1], F32, name="istd3", tag="istd3")
        nc.vector.reciprocal(istd3[:], sq3[:])
        nm3 = spool.tile([P, 1], F32, name="nm3", tag="nm3")
        nc.vector.scalar_tensor_tensor(nm3[:], in0=mv3[:, 0:1], scalar=-1.0,
                                       in1=istd3[:], op0=OP.mult, op1=OP.mult)
        ofin = zcpool.tile([P, B], F32, name="ofin", tag="ofin")
        nc.vector.tensor_scalar(out=ofin[:], in0=zfin[:], scalar1=istd3[:],
                                scalar2=nm3[:], op0=OP.mult, op1=OP.add)
        nc.sync.dma_start(outT[:], ofin[:])

    nc.compile()
    return nc


def _pack_inputs(features, w1, b1, w2, b2, w3, b3,
                 rows1, cols1, rows2, cols2, rows3, cols3):
    """Host-side packing into per-core contiguous [128, N] tile layouts."""
    f32 = np.float32
    features = np.asarray(features, f32)
    w1 = np.asarray(w1, f32); b1 = np.asarray(b1, f32)
    w2 = np.asarray(w2, f32); b2 = np.asarray(b2, f32)
    w3 = np.asarray(w3, f32)
    rows1 = np.asarray(rows1); cols1 = np.asarray(cols1)
    rows2 = np.asarray(rows2); cols2 = np.asarray(cols2)
    rows3 = np.asarray(rows3); cols3 = np.asarray(cols3)

    w1r = np.empty(HID, f32); w1r[rows1] = w1
    b1r = np.empty(HID, f32); b1r[rows1] = b1
    c1r = np.empty(HID, np.int64); c1r[rows1] = cols1

    order2 = np.argsort(rows2, kind="stable")
    r2 = rows2[order2]; c2 = cols2[order2]; v2 = w2[order2]

    W3d = np.zeros((HID, N_TF), f32)
    np.add.at(W3d, (cols3.astype(np.int64), rows3.astype(np.int64)), w3)

    featT = np.ascontiguousarray(features.T)  # [N_GENES, B]
    in_maps = []
    for c in range(NCORES):
        hbase = c * HSH
        # xrd[p, t*B+b] = w1[h]*features[b, gene(h)] + b1[h],  h = hbase+t*128+p
        genes = c1r[hbase:hbase + HSH]                      # [HSH]
        xg = featT[genes] * w1r[hbase:hbase + HSH, None] + b1r[hbase:hbase + HSH, None]
        xrep = xg.reshape(NT, P, B).transpose(1, 0, 2).reshape(P, NT * B)

        w2t = np.zeros((NT, P, P), f32)
        for t in range(NT):
            R0 = hbase + t * P
            es = slice(WM * R0, WM * (R0 + P))
            np.add.at(w2t[t], (c2[es] - R0, r2[es] - R0), v2[es])

        w3t = W3d[hbase:hbase + HSH].reshape(NT, P, N_TF)

        in_maps.append({
            "xrd": np.ascontiguousarray(xrep).astype(BF16),
            "w2d": np.ascontiguousarray(w2t.transpose(1, 0, 2).reshape(P, NT * P)).astype(BF16),
            "w3d": np.ascontiguousarray(w3t.transpose(1, 0, 2).reshape(P, NT * N_TF)).astype(BF16),
            "b2d": np.ascontiguousarray(b2[hbase:hbase + HSH].reshape(NT, P).T),
        })
    return in_maps


def kernel(**inputs) -> np.ndarray:
    global LAST_RESULT
    if "nc" not in _cache:
        _cache["nc"] = _build_graph()
    nc = _cache["nc"]

    in_maps = _pack_inputs(**inputs)
    # b3 is dropped: BN3 subtracts the per-TF batch mean, so a per-TF constant
    # bias cancels exactly.

    res = bass_utils.run_bass_kernel_spmd(
        nc, in_maps, core_ids=list(range(NCORES)), trace=TRACE)
    LAST_RESULT = res

    outT = np.concatenate([res.results[c]["outT"] for c in range(NCORES)], axis=0)
    return np.ascontiguousarray(outT.T.astype(np.float32))


# revision 48
# speedup vs baseline: 1.0607x; 1.0607x over previous
"""Trainium2 Bass kernel: sparse AE encoder (L1 fan-in-1 -> relu/BN -> L2 block-diag
4x4 -> relu/BN -> L3 sparse 256-nnz/TF -> BN), SPMD over 8 NeuronCores.

Sharding: hidden axis across cores (BN1/BN2 local; one z-reduce at the end).
Host packs w1*x+b1 into the hidden-row x layout so L1 is a bare batched relu.
L2 runs on the TensorEngine with block-diagonal [128,128] bf16 stationaries,
L3 as dense matmul against the host-densified W3 shard (batch-half stationary,
w3 streaming, N=512 PSUM accumulators over all 32 tiles).

Schedule notes (v14):
- PE warmup burst (identity matmuls) during the input-DMA phase nudges the
  HAM clock gate toward 2.4 GHz before real matmuls start.
- norm_params does the bn_stats half-combine on [128,8] groups (GB=8), with
  1/sqrt as Scalar-Sqrt + Vector-reciprocal (2 ops instead of a 6-op Newton
  chain).  h1n normalize on GpSimd, h2n split Scalar/Vector using
  relu(istd*h) == istd*h (h = relu output >= 0, istd > 0) so Scalar's Relu
  with a scale AP does the scale-only normalize.  NEVER split one pass
  across Vector AND GpSimd: DVE 2-port mode locks GpSimd out of their shared
  SBUF port (measured 3.9us per op vs 560ns).
- ONE AllToAll for the z partials (batch-major layout, 2 PE transposes after
  the local tree-reduce instead of 16 before).  The collective firmware only
  becomes responsive ~60-67us after kernel start and serializes collectives,
  so chunked/pipelined exchanges buy nothing; a tiny sacrificial AllToAll
  (with its input actually written - an unwritten input makes the Tile
  scheduler classify the collective as never-ready and emit it LAST) absorbs
  the ncfw wake + cold-mesh pass while compute runs.
"""

import numpy as np
import ml_dtypes

import concourse.bacc as bacc
import concourse.bass as bass
import concourse.tile as tile
import concourse.mybir as mybir
from concourse import bass_utils
from concourse.masks import make_identity

N_GENES = 8192
WM = 4
HID = N_GENES * WM          # 32768
N_TF = 1024
B = 256
EPS = 1e-5

NCORES = 8
HSH = HID // NCORES         # 4096 hidden rows / core
P = 128
NT = HSH // P               # 32 hidden tiles / core
GB = 8                      # group size (tiles)
NG = NT // GB               # 4 groups
HT = NT // 2                # 16 tiles per z-chunk

BF16 = ml_dtypes.bfloat16
F32 = mybir.dt.float32
F16 = mybir.dt.float16
BF = mybir.dt.bfloat16
I32 = mybir.dt.int32
AF = mybir.ActivationFunctionType
OP = mybir.AluOpType

TRACE = False
LAST_RESULT = None

_cache = {}


def _build_graph():
    nc = bacc.Bacc("TRN2", target_bir_lowering=False, debug=False, num_devices=NCORES)

    xrd = nc.dram_tensor("xrd", [P, NT * B], BF, kind="ExternalInput").ap()
    w2d = nc.dram_tensor("w2d", [P, NT * P], BF, kind="ExternalInput").ap()
    w3d = nc.dram_tensor("w3d", [P, NT * N_TF], BF, kind="ExternalInput").ap()
    b2d = nc.dram_tensor("b2d", [P, NT], F32, kind="ExternalInput").ap()
    outT = nc.dram_tensor("outT", [P, B], F32, kind="ExternalOutput").ap()

    from contextlib import ExitStack
    with tile.TileContext(nc) as tc, ExitStack() as ctx:
        cpool = ctx.enter_context(tc.tile_pool(name="const", bufs=1))
        wpool = ctx.enter_context(tc.tile_pool(name="wts", bufs=1))
        apool = ctx.enter_context(tc.tile_pool(name="acts", bufs=1))
        spool = ctx.enter_context(tc.tile_pool(name="stats", bufs=4))
        zcpool = ctx.enter_context(tc.tile_pool(name="zchunk", bufs=1))
        psZp = ctx.enter_context(tc.tile_pool(name="psZ", bufs=1, space="PSUM"))
        psLp = ctx.enter_context(tc.tile_pool(name="psL", bufs=2, space="PSUM"))
        psWp = ctx.enter_context(tc.tile_pool(name="psW", bufs=1, space="PSUM"))
        psTp = ctx.enter_context(tc.tile_pool(name="psT", bufs=1, space="PSUM"))
        dpool = ctx.enter_context(tc.tile_pool(name="dram", bufs=1, space="DRAM"))

        # ---- warmup collective (slot #2 after the bass prelude barrier):
        # absorbs the ncfw wakeup + cold-mesh pass while compute runs, so the
        # real AllToAll runs warm.  Its input must actually be written or the
        # Tile scheduler emits the collective last.
        dum_in = dpool.tile([NCORES, NCORES], F16, name="dum_in")
        dum_out = dpool.tile([NCORES, NCORES], F16, name="dum_out")
        dum_sb = cpool.tile([NCORES, NCORES], F16, name="dum_sb")
        nc.gpsimd.memset(dum_sb[:], 0)
        nc.sync.dma_start(dum_in[:], dum_sb[:])
        nc.gpsimd.collective_compute(
            "AllToAll", OP.bypass, replica_groups=[list(range(NCORES))],
            ins=[dum_in.opt()], outs=[dum_out.opt()])

        # ---- static loads, issued in compute-consumption order -----------
        b2t = cpool.tile([P, NT], F32, name="b2t")
        nc.sync.dma_start(b2t[:], b2d[:])

        # Force the sqrt table set to load up front: Relu/Copy are filler
        # functions present in every set, so after this single load no other
        # ACT_TABLE_LOAD is needed.
        sqwt = cpool.tile([P, 1], F32, name="sqwt")
        nc.gpsimd.memset(sqwt[:], 1.0)
        nc.scalar.activation(sqwt[:], sqwt[:], AF.Sqrt)

        xrep = wpool.tile([P, NT * B], BF, name="xrep")
        w2s = wpool.tile([P, NT * P], BF, name="w2s")
        w3s = wpool.tile([P, NT * N_TF], BF, name="w3s")

        def load_x(c):     # 4 chunks of 8 tiles each (512KB)
            xcw = GB * B
            nc.sync.dma_start(xrep[:, c * xcw:(c + 1) * xcw],
                              xrd[:, c * xcw:(c + 1) * xcw])

        def load_w2(c):    # 2 chunks of 16 tiles each (512KB)
            cw = 16 * P
            nc.sync.dma_start(w2s[:, c * cw:(c + 1) * cw],
                              w2d[:, c * cw:(c + 1) * cw])

        def load_w3(c):    # 8 chunks of 4 tiles each (1MB)
            cw = 4 * N_TF
            nc.sync.dma_start(w3s[:, c * cw:(c + 1) * cw],
                              w3d[:, c * cw:(c + 1) * cw])

        load_x(0); load_w2(0); load_x(1)
        load_w3(0); load_w3(1)
        load_x(2); load_w2(1); load_w3(2); load_w3(3)
        load_x(3); load_w3(4); load_w3(5)
        load_w3(6); load_w3(7)

        idt = cpool.tile([P, P], F16, name="idt")
        make_identity(nc, idt[:])

        # ---- PE warmup burst: hold the HAM clock gate at 8/8 -------------
        # The PE is idle during the input-DMA phase; a stream of junk
        # identity matmuls warms the clock gate (~3.4us busy window) so the
        # real L2/L3 matmuls run at 2.4 GHz instead of 1.2.
        NWARM = 56
        psW = psWp.tile([P, P], F32, name="psW", tag="psW")
        for i in range(NWARM):
            nc.tensor.matmul(psW[:], lhsT=idt[:], rhs=idt[:],
                             start=True, stop=True, skip_group_check=True)

        def rsqrt_sv(istd, var, w=GB):
            """istd = 1/sqrt(var): Sqrt on Scalar (spline) + reciprocal on
            Vector (full-precision HW divide) — 2 ops instead of a 6-op
            Newton chain on Vector."""
            sq = spool.tile([P, GB], F32, name="sq", tag="sq")
            nc.scalar.activation(sq[:, :w], var, AF.Sqrt)
            nc.vector.reciprocal(istd, sq[:, :w])

        hrA = apool.tile([P, NT * B], BF, name="hrA")
        h1n = apool.tile([P, NT * B], BF, name="h1n")
        hrB = apool.tile([P, NT * B], BF, name="hrB")
        h2n = apool.tile([P, NT * B], BF, name="h2n")
        st1 = apool.tile([P, NT * 6], F32, name="st1")
        st2 = apool.tile([P, NT * 6], F32, name="st2")

        # layer-3 psums, accumulated across HT tiles per chunk
        psZ = [[psZp.tile([P, 512], F32, name=f"psZ{bh}{th}", tag=f"psZ{bh}{th}")
                for th in range(2)] for bh in range(2)]

        def norm_params(st, g0, istd, nm):
            """bn_stats 6-tuples (even/odd column halves) -> istd [, nm]."""
            sv = st[:, g0 * 6:(g0 + GB) * 6].rearrange("p (t s) -> p t s", s=6)
            me, mo = sv[:, :, 1], sv[:, :, 4]
            M2e, M2o = sv[:, :, 2], sv[:, :, 5]
            dm = spool.tile([P, GB], F32, name="dm", tag="dm")
            nc.vector.tensor_tensor(dm[:], me, mo, op=OP.subtract)
            vq = spool.tile([P, GB], F32, name="vq", tag="vq")
            nc.vector.scalar_tensor_tensor(vq[:], in0=dm[:], scalar=0.25,
                                           in1=dm[:], op0=OP.mult, op1=OP.mult)
            var = spool.tile([P, GB], F32, name="var", tag="var")
            nc.vector.tensor_tensor(var[:], M2e, M2o, op=OP.add)
            nc.vector.tensor_scalar(out=var[:], in0=var[:], scalar1=1.0 / B,
                                    scalar2=EPS, op0=OP.mult, op1=OP.add)
            nc.vector.tensor_tensor(var[:], var[:], vq[:], op=OP.add)
            rsqrt_sv(istd[:], var[:])
            if nm is not None:
                ms = spool.tile([P, GB], F32, name="ms", tag="ms")
                nc.vector.tensor_tensor(ms[:], me, mo, op=OP.add)
                nc.vector.scalar_tensor_tensor(nm[:], in0=ms[:], scalar=-0.5,
                                               in1=istd[:], op0=OP.mult,
                                               op1=OP.mult)

        def emitA(g):
            """L1 relu (Scalar, batched, pair-interleaved out) + BN1 stats
            (Vector, one per pair) + normalize (GpSimd/Vector split)."""
            t0 = g * GB
            for h in range(2):  # two [128, 4*B] relus per group
                lo = (t0 + 4 * h) * B
                nc.scalar.activation(hrA[:, lo:lo + 4 * B],
                                     xrep[:, lo:lo + 4 * B], AF.Relu)
            for t in range(t0, t0 + GB):
                nc.vector.bn_stats(st1[:, t * 6:(t + 1) * 6],
                                   hrA[:, t * B:(t + 1) * B])
            istd = spool.tile([P, GB], F32, name="istdA", tag="istdA")
            nm = spool.tile([P, GB], F32, name="nmA", tag="nmA")
            norm_params(st1, t0, istd, nm)
            for t in range(t0, t0 + GB):
                j = t - t0
                nc.gpsimd.tensor_scalar(out=h1n[:, t * B:(t + 1) * B],
                                        in0=hrA[:, t * B:(t + 1) * B],
                                        scalar1=istd[:, j:j + 1],
                                        scalar2=nm[:, j:j + 1],
                                        op0=OP.mult, op1=OP.add)

        def emitB_front(g):
            """L2 matmul (PE) + relu (Scalar, pair-interleaved out) + BN2
            stats (Vector, one per pair)."""
            t0 = g * GB
            for t in range(t0, t0 + GB):
                if t % 2 == 0:  # one PSUM bank holds two tiles' L2 outputs
                    ps2 = psLp.tile([P, 2 * B], F32, name="psL", tag="psL")
                ps = ps2[:, (t % 2) * B:(t % 2 + 1) * B]
                nc.tensor.matmul(ps, lhsT=w2s[:, t * P:(t + 1) * P],
                                 rhs=h1n[:, t * B:(t + 1) * B],
                                 start=True, stop=True, skip_group_check=True)
                nc.scalar.activation(hrB[:, t * B:(t + 1) * B], ps, AF.Relu,
                                     bias=b2t[:, t:t + 1])
                nc.vector.bn_stats(st2[:, t * 6:(t + 1) * 6],
                                   hrB[:, t * B:(t + 1) * B])

        def emitB_norm(g):
            """BN2 scale-only normalize (mean shift cancels in BN3)."""
            t0 = g * GB
            istd = spool.tile([P, GB], F32, name="istdB", tag="istdB")
            norm_params(st2, t0, istd, None)
            for t in range(t0, t0 + GB):
                j = t - t0
                if t % 2 == 0:
                    # relu(istd*h) == istd*relu(h) == istd*h for h = relu out,
                    # istd > 0 — lets Scalar do the scale-only normalize.
                    nc.scalar.activation(h2n[:, t * B:(t + 1) * B],
                                         hrB[:, t * B:(t + 1) * B], AF.Relu,
                                         scale=istd[:, j:j + 1])
                else:
                    nc.vector.tensor_scalar(out=h2n[:, t * B:(t + 1) * B],
                                            in0=hrB[:, t * B:(t + 1) * B],
                                            scalar1=istd[:, j:j + 1],
                                            scalar2=None, op0=OP.mult)

        # z accumulation chunk boundaries: first chunk is small (group 0
        # only) so its AllToAll triggers early and absorbs the cold-mesh cost
        # under compute; the later two run on a warm mesh.
        CH_START = {0}
        CH_STOP = {31}

        def emitL3(lo, hi):
            """z accumulation for tiles [lo, hi)."""
            for t in range(lo, hi):
                for bh in range(2):
                    for th in range(2):
                        nc.tensor.matmul(
                            psZ[bh][th][:],
                            lhsT=h2n[:, t * B + bh * P: t * B + (bh + 1) * P],
                            rhs=w3s[:, t * N_TF + th * 512: t * N_TF + (th + 1) * 512],
                            start=(t in CH_START), stop=(t in CH_STOP),
                            skip_group_check=True)

        # DRAM tensors for the two chunked AllToAlls.
        # zin layout: [j(rank) 8, bh 2, p 128, t 128] f16 -> rank j's slice is
        # rows [256j, 256j+256) = [bh*128+p, t].
        zin = [dpool.tile([NCORES * B, P], F16, name=f"zin{c}") for c in range(1)]
        za = [dpool.tile([NCORES * B, P], F16, name=f"za{c}") for c in range(1)]

        def drain_chunk(c):
            """psZ -> SBUF f16 -> DRAM zin[c] -> AllToAll trigger."""
            zc = zcpool.tile([P, 2 * N_TF], F16, name="zc", tag="zc")
            # zc columns: (bh, th*512+t~): Scalar does bh=0, Vector bh=1
            for th in range(2):
                nc.scalar.activation(zc[:, th * 512:(th + 1) * 512],
                                     psZ[0][th][:], AF.Copy)
                nc.scalar.activation(zc[:, N_TF + th * 512: N_TF + (th + 1) * 512],
                                     psZ[1][th][:], AF.Copy)
            zv = zin[c].rearrange("(j bh p) t -> p j bh t", j=NCORES, bh=2)
            for bh in range(2):
                nc.sync.dma_start(
                    zv[:, :, bh, :],
                    zc[:, bh * N_TF:(bh + 1) * N_TF].rearrange(
                        "p (j t) -> p j t", j=NCORES))
            nc.gpsimd.collective_compute(
                "AllToAll", OP.bypass, replica_groups=[list(range(NCORES))],
                ins=[zin[c].opt()], outs=[za[c].opt()])

        def reduce_chunk(c, acc_prev):
            """za[c] -> SBUF, tree-reduce the 16 received [128,128] blocks."""
            zs = zcpool.tile([P, NCORES * B], F16, name=f"zsum{c}", tag=f"zsum{c}")
            nc.sync.dma_start(
                zs[:].rearrange("p (q t) -> p q t", q=2 * NCORES),
                za[c].rearrange("(q p) t -> p q t", p=P))
            zv = zs[:].rearrange("p (r c) -> p r c", r=NCORES)
            t4 = zcpool.tile([P, 4 * B], F16, name=f"t4_{c}", tag="t4")
            for k in range(4):
                nc.vector.tensor_tensor(t4[:, k * B:(k + 1) * B],
                                        zv[:, 2 * k, :], zv[:, 2 * k + 1, :],
                                        op=OP.add)
            t2 = zcpool.tile([P, 2 * B], F16, name=f"t2_{c}", tag="t2")
            nc.vector.tensor_tensor(t2[:], t4[:, 0:2 * B], t4[:, 2 * B:4 * B],
                                    op=OP.add)
            red = zcpool.tile([P, B], F16, name=f"red{c}", tag=f"red{c}")
            nc.vector.tensor_tensor(red[:], t2[:, 0:B], t2[:, B:2 * B], op=OP.add)
            if acc_prev is None:
                return red
            out = zcpool.tile([P, B], F16, name=f"redf{c}", tag=f"redf{c}")
            nc.vector.tensor_tensor(out[:], red[:], acc_prev[:], op=OP.add)
            return out

        # ---- main pipeline ------------------------------------------------
        emitA(0)
        emitB_front(0); emitB_norm(0)
        emitA(1)
        emitB_front(1); emitB_norm(1)
        emitL3(0, 8)
        emitA(2)
        emitB_front(2); emitB_norm(2)
        emitL3(8, 16)
        emitA(3)
        emitB_front(3); emitB_norm(3)
        emitL3(16, 24)
        emitL3(24, 32)
        drain_chunk(0)
        redf = reduce_chunk(0, None)

        # ---- transpose to TF-major, BN3, output ---------------------------
        zfin = zcpool.tile([P, B], F16, name="zfin", tag="zfin")
        for bh in range(2):
            pst = psTp.tile([P, P], F16, name="pst", tag="pst")
            nc.tensor.transpose(pst[:], in_=redf[:, bh * P:(bh + 1) * P],
                                identity=idt[:])
            nc.scalar.activation(zfin[:, bh * P:(bh + 1) * P], pst[:], AF.Copy)

        st6 = spool.tile([P, 6], F32, name="st6", tag="st6")
        nc.vector.bn_stats(st6[:], zfin[:])
        mv3 = spool.tile([P, 2], F32, name="mv3", tag="mv3")
        nc.vector.bn_aggr(mv3[:], st6[:])
        var3 = spool.tile([P, 1], F32, name="var3", tag="var3")
        nc.vector.tensor_scalar(out=var3[:], in0=mv3[:, 1:2], scalar1=1.0,
                                scalar2=EPS, op0=OP.mult, op1=OP.add)
        istd3 = spool.tile([P, 1], F32, name="istd3", tag="istd3")
        rsqrt_sv(istd3[:], var3[:], w=1)
        nm3 = spool.tile([P, 1], F32, name="nm3", tag="nm3")
        nc.vector.scalar_tensor_tensor(nm3[:], in0=mv3[:, 0:1], scalar=-1.0,
                                       in1=istd3[:], op0=OP.mult, op1=OP.mult)
        ofin = zcpool.tile([P, B], F32, name="ofin", tag="ofin")
        nc.vector.tensor_scalar(out=ofin[:], in0=zfin[:], scalar1=istd3[:],
                                scalar2=nm3[:], op0=OP.mult, op1=OP.add)
        nc.sync.dma_start(outT[:], ofin[:])

    nc.compile()
    return nc


def _pack_inputs(features, w1, b1, w2, b2, w3, b3,
                 rows1, cols1, rows2, cols2, rows3, cols3):
    """Host-side packing into per-core contiguous [128, N] tile layouts."""
    f32 = np.float32
    features = np.asarray(features, f32)
    w1 = np.asarray(w1, f32); b1 = np.asarray(b1, f32)
    w2 = np.asarray(w2, f32); b2 = np.asarray(b2, f32)
    w3 = np.asarray(w3, f32)
    rows1 = np.asarray(rows1); cols1 = np.asarray(cols1)
    rows2 = np.asarray(rows2); cols2 = np.asarray(cols2)
    rows3 = np.asarray(rows3); cols3 = np.asarray(cols3)

    w1r = np.empty(HID, f32); w1r[rows1] = w1
    b1r = np.empty(HID, f32); b1r[rows1] = b1
    c1r = np.empty(HID, np.int64); c1r[rows1] = cols1

    order2 = np.argsort(rows2, kind="stable")
    r2 = rows2[order2]; c2 = cols2[order2]; v2 = w2[order2]

    W3d = np.zeros((HID, N_TF), f32)
    np.add.at(W3d, (cols3.astype(np.int64), rows3.astype(np.int64)), w3)

    featT = np.ascontiguousarray(features.T)  # [N_GENES, B]
    in_maps = []
    for c in range(NCORES):
        hbase = c * HSH
        # xrd[p, t*B+b] = w1[h]*features[b, gene(h)] + b1[h],  h = hbase+t*128+p
        genes = c1r[hbase:hbase + HSH]                      # [HSH]
        xg = featT[genes] * w1r[hbase:hbase + HSH, None] + b1r[hbase:hbase + HSH, None]
        xrep = xg.reshape(NT, P, B).transpose(1, 0, 2).reshape(P, NT * B)

        w2t = np.zeros((NT, P, P), f32)
        for t in range(NT):
            R0 = hbase + t * P
            es = slice(WM * R0, WM * (R0 + P))
            np.add.at(w2t[t], (c2[es] - R0, r2[es] - R0), v2[es])

        w3t = W3d[hbase:hbase + HSH].reshape(NT, P, N_TF)

        in_maps.append({
            "xrd": np.ascontiguousarray(xrep).astype(BF16),
            "w2d": np.ascontiguousarray(w2t.transpose(1, 0, 2).reshape(P, NT * P)).astype(BF16),
            "w3d": np.ascontiguousarray(w3t.transpose(1, 0, 2).reshape(P, NT * N_TF)).astype(BF16),
            "b2d": np.ascontiguousarray(b2[hbase:hbase + HSH].reshape(NT, P).T),
        })
    return in_maps


def kernel(**inputs) -> np.ndarray:
    global LAST_RESULT
    if "nc" not in _cache:
        _cache["nc"] = _build_graph()
    nc = _cache["nc"]

    in_maps = _pack_inputs(**inputs)
    # b3 is dropped: BN3 subtracts the per-TF batch mean, so a per-TF constant
    # bias cancels exactly.

    res = bass_utils.run_bass_kernel_spmd(
        nc, in_maps, core_ids=list(range(NCORES)), trace=TRACE)
    LAST_RESULT = res

    outT = np.concatenate([res.results[c]["outT"] for c in range(NCORES)], axis=0)
    return np.ascontiguousarray(outT.T.astype(np.float32))
